# revision 11
# baseline (speedup 1.0000x reference)
"""Causal attention (B=4, S=2048, D=1024) on 8 Trainium2 NeuronCores.

Sharding: core c = (batch b = c//2, parity h = c%2). Each core owns the
1024 query rows of batch b with (q // 64) % 2 == h, sorted ascending
(64-row interleave). This makes the causal work profile identical on
every core: local query chunk qc (128 rows) spans global 128-row classes
2qc and 2qc+1, so it needs exactly the key-block prefix 0..2qc+1. All
cores run the same SPMD program with per-chunk key-block counts
C = [2,4,...,16] (72 score blocks/core instead of 128 unmasked), and the
per-batch work is exactly balanced between the two cores.

The program is specialized on the observed mask block structure
(computed on host in make_in_maps, cached per structure): fully-kept
score blocks skip masking entirely; partially-masked blocks multiply
exp(scores) by a {0,1} bf16 mask tile (SBUF-resident, 1MB/core vs the
33MB fp32 additive-bias tensor this replaces). Arbitrary masks fall
back to C=[16,16,16,16] with per-block masking - always correct, just
slower.

All inputs are converted to bf16 and pre-shuffled on the host into the
exact SBUF layout each consumer reads, so every load is one contiguous
[128, N] DMA with no on-chip staging copies, and all three weight
matrices fit in SBUF simultaneously and are prefetched up front (the
phase-boundary stalls of the staged variant came from weight loads
having to wait for the previous phase's SBUF space to free).

Per-core kernel (SPMD, identical program, per-core data):
  Phase A: fused K^T/V projection streaming x^T key-chunks (V one chunk
           behind K so the V weights' arrival is off the critical path),
           then Q^T. All outputs bf16, SBUF-resident.
  Phase B: transposed-scores flash attention, software-pipelined so the
           PE never waits on the exp/mask chain: score matmuls for block
           i+2 are issued between the probability consumption (l/O
           matmuls) of blocks i-1 and i. S^T = K @ Q^T, exp on ScalarE
           (no max subtraction; scores are ~N(0,1) by construction),
           P in bf16, O = P @ V and l = P^T-column sums accumulated in
           PSUM across the block prefix, then normalize O by 1/l
           (split across DVE and ACT) and DMA out.
"""

import sys

sys.path.insert(0, "/opt/trn_rl_repo")

import numpy as np
import ml_dtypes

import concourse.bass as bass
import concourse.mybir as mybir
from concourse import tile
from concourse.bass_utils import run_bass_kernel_spmd

F32 = mybir.dt.float32
BF16 = mybir.dt.bfloat16
AF = mybir.ActivationFunctionType
BF = ml_dtypes.bfloat16

B, S, D = 4, 2048, 1024
NQ = 1024          # query rows per core
NKB = 16           # key blocks of 128
NQC = 8            # query column chunks per core
QW = 128           # query width of one score tile
NMC = 8            # d_model chunks of 128 (contraction)
NDC = 8            # d_k chunks of 128
SKC = 8            # streamed key chunks of 256 in phase A
DV = 512           # v-column tile width
NDV = D // DV
NQB = QW // 128
SCALE = 1.0 / 32.0  # 1/sqrt(D_K)

# Local row l of core parity h <-> global query row (l//64)*128 + h*64 + l%64.
_LROWS = (np.arange(NQ) // 64) * 128 + (np.arange(NQ) % 64)


def _rows(h):
    return _LROWS + h * 64


# Causal structure: chunk qc (128 rows = classes 2qc,2qc+1) needs key
# blocks 0..2qc+1; the two diagonal-straddling blocks are partially masked.
_CAUSAL = (
    tuple(2 * qc + 2 for qc in range(NQC)),
    frozenset((qc, j) for qc in range(NQC) for j in range(2 * qc, 2 * qc + 2)),
)


def _mask_structure(mask):
    """(C, mixed) uniform across all 8 cores for the observed mask.

    C[qc]: number of key blocks (prefix 0..C-1) chunk qc computes.
    mixed: (qc, j) blocks that are not all-True on every core and thus
    get an explicit multiplicative mask tile.
    Requires every needed key block to sit in a prefix; C=16 everywhere
    is the always-valid fallback.
    """
    alls = np.ones((NQC, NKB), bool)
    anys = np.zeros((NQC, NKB), bool)
    for b in range(B):
        for h in range(2):
            m = mask[b][_rows(h)]  # [1024, 2048]
            mr = m.reshape(NQC, QW, NKB, 128)
            alls &= mr.all(axis=(1, 3))
            anys |= mr.any(axis=(1, 3))
    C = []
    for qc in range(NQC):
        need = np.nonzero(anys[qc])[0]
        C.append(int(need[-1]) + 1 if len(need) else 1)
    mixed = frozenset(
        (qc, j) for qc in range(NQC) for j in range(C[qc]) if not alls[qc, j]
    )
    return (tuple(C), mixed)


def _mix_order(structure):
    C, mixed = structure
    return [(qc, j) for qc in range(NQC) for j in range(C[qc]) if (qc, j) in mixed]


def _build_nc(structure):
    C, mixed = structure
    nmix = max(1, len(mixed))
    mix_index = {qj: i for i, qj in enumerate(_mix_order(structure))}

    nc = bass.Bass()
    # All inputs bf16, host-pre-shuffled into exact SBUF layouts.
    xq3 = nc.declare_dram_parameter("xq3", [NMC, 128, NQ], BF16, isOutput=False)
    xkv3 = nc.declare_dram_parameter("xkv3", [SKC, 128, NMC * 256], BF16, isOutput=False)
    wq3 = nc.declare_dram_parameter("wq3", [NDC, 128, NMC * 128], BF16, isOutput=False)
    wk3 = nc.declare_dram_parameter("wk3", [NDC, 128, NMC * 128], BF16, isOutput=False)
    wv3 = nc.declare_dram_parameter("wv3", [NDV, 128, NMC * DV], BF16, isOutput=False)
    mmh = nc.declare_dram_parameter("mmh", [128, nmix * QW], BF16, isOutput=False)
    out = nc.declare_dram_parameter("out", [NQ, D], BF16, isOutput=True)

    with tile.TileContext(nc) as tc:
        with tc.tile_pool(name="res", bufs=1) as res, \
             tc.tile_pool(name="xcp", bufs=2) as xcp, \
             tc.tile_pool(name="psp", bufs=1, space="PSUM") as psp:
            # Resident: Q^T [p=dk, dc, q]; K^T [p=dk, dc, k]; V [p=k, kb, dv].
            qt_sb = res.tile([128, NDC * NQ], BF16, name="qt_sb")
            kt_sb = res.tile([128, NDC * S], BF16, name="kt_sb")
            v_sb = res.tile([128, NKB * D], BF16, name="v_sb")
            wq_sb = res.tile([128, NDC * NMC * 128], BF16, name="wq_sb")
            wk_sb = res.tile([128, NDC * NMC * 128], BF16, name="wk_sb")
            wv_sb = res.tile([128, NDV * NMC * DV], BF16, name="wv_sb")
            xq_sb = res.tile([128, NMC * NQ], BF16, name="xq_sb")
            mm_sb = res.tile([128, nmix * QW], BF16, name="mm_sb")
            ones = res.tile([128, 1], BF16, name="ones")
            nc.vector.memset(ones[:], 1.0)

            # DMA issue order == need order: the cost of every transfer is
            # serialized on the global DMA engines, so first-needed-first.
            xcols = {}

            def fetch_xcol(kc):
                xcol = xcp.tile([128, NMC * 256], BF16, name="xcol", tag="xcol", bufs=3)
                nc.sync.dma_start(xcol[:], xkv3[kc])
                xcols[kc] = xcol

            nc.sync.dma_start(wk_sb[:, 0:1024], wk3[0])
            fetch_xcol(0)
            nc.sync.dma_start(wk_sb[:, 1024:2048], wk3[1])
            nc.sync.dma_start(wk_sb[:, 2048:3072], wk3[2])
            fetch_xcol(1)
            for dc in range(3, NDC):
                nc.sync.dma_start(wk_sb[:, dc * 1024:(dc + 1) * 1024], wk3[dc])
            for dvc in range(NDV):
                nc.sync.dma_start(wv_sb[:, dvc * 4096:(dvc + 1) * 4096], wv3[dvc])

            # ---- Phase A: fused K^T / V projection (V one chunk behind K),
            # then Q^T. K^T = Wk @ x^T; V = x @ Wv^T; Q^T = Wq @ x_q^T.
            for kc in range(SKC + 1):
                if kc < SKC:
                    xcol = xcols[kc]
                    for dc in range(NDC):
                        ps = psp.tile([128, 512], F32, name="psk", tag="st", bufs=3)[:, :256]
                        for mc in range(NMC):
                            nc.tensor.matmul(
                                ps[:],
                                lhsT=wk_sb[:, dc * 1024 + mc * 128: dc * 1024 + mc * 128 + 128],
                                rhs=xcol[:, mc * 256:(mc + 1) * 256],
                                start=(mc == 0),
                                stop=(mc == NMC - 1),
                            )
                        nc.vector.tensor_copy(
                            kt_sb[:, dc * S + kc * 256: dc * S + kc * 256 + 256], ps[:]
                        )
                if kc >= 1:
                    vcol = xcols.pop(kc - 1)
                    for kbl in range(2):
                        kb = (kc - 1) * 2 + kbl
                        for dvc in range(NDV):
                            ps = psp.tile([128, 512], F32, name="psv", tag="st", bufs=3)[:, :DV]
                            for mc in range(NMC):
                                nc.tensor.matmul(
                                    ps[:],
                                    lhsT=vcol[:, mc * 256 + kbl * 128: mc * 256 + kbl * 128 + 128],
                                    rhs=wv_sb[:, dvc * 4096 + mc * DV: dvc * 4096 + mc * DV + DV],
                                    start=(mc == 0),
                                    stop=(mc == NMC - 1),
                                )
                            nc.scalar.activation(
                                v_sb[:, kb * D + dvc * DV: kb * D + dvc * DV + DV],
                                ps[:], AF.Copy,
                            )
                if kc + 2 < SKC:
                    fetch_xcol(kc + 2)
            # Late prefetches: issued behind all x-column traffic, land long
            # before phase A ends.
            for mc in range(NMC):
                nc.sync.dma_start(xq_sb[:, mc * NQ:(mc + 1) * NQ], xq3[mc])
            for dc in range(NDC):
                nc.sync.dma_start(wq_sb[:, dc * 1024:(dc + 1) * 1024], wq3[dc])
            nc.sync.dma_start(mm_sb[:], mmh[:])
            for dc in range(NDC):
                for q2 in range(NQ // 512):
                    ps = psp.tile([128, 512], F32, name="psq", tag="st", bufs=3)
                    for mc in range(NMC):
                        nc.tensor.matmul(
                            ps[:],
                            lhsT=wq_sb[:, dc * 1024 + mc * 128: dc * 1024 + mc * 128 + 128],
                            rhs=xq_sb[:, mc * NQ + q2 * 512: mc * NQ + q2 * 512 + 512],
                            start=(mc == 0),
                            stop=(mc == NMC - 1),
                        )
                    nc.vector.tensor_copy(
                        qt_sb[:, dc * NQ + q2 * 512: dc * NQ + q2 * 512 + 512], ps[:]
                    )

            # ---------------- Phase B: attention ----------------
            with (
                tc.tile_pool(name="pap", bufs=3) as pap,
                tc.tile_pool(name="pep", bufs=3) as pep,
                tc.tile_pool(name="otp", bufs=4) as otp,
                tc.tile_pool(name="rcp", bufs=2) as rcp,
            ):
                blocks = [(qc, j) for qc in range(NQC) for j in range(C[qc])]
                NB = len(blocks)

                def emit_sc_probs(bi):
                    # Score matmuls + exp (+ mask) for block bi; returns the
                    # bf16 probability tile consumed by the l/O matmuls.
                    qc, j = blocks[bi]
                    st = psp.tile([128, 512], F32, name="st", tag="st", bufs=3)[:, :QW]
                    for dc in range(NDC):
                        nc.tensor.matmul(
                            st[:],
                            lhsT=kt_sb[:, dc * S + j * 128: dc * S + j * 128 + 128],
                            rhs=qt_sb[:, dc * NQ + qc * QW: dc * NQ + qc * QW + QW],
                            start=(dc == 0),
                            stop=(dc == NDC - 1),
                        )
                    pe = pep.tile([128, QW], BF16, name="pe", tag="pe")
                    if (qc, j) in mix_index:
                        mi = mix_index[(qc, j)]
                        pa = pap.tile([128, QW], BF16, name="pa", tag="pa")
                        nc.scalar.activation(pa[:], st[:], AF.Exp, scale=SCALE)
                        nc.vector.tensor_mul(
                            pe[:], pa[:], mm_sb[:, mi * QW:(mi + 1) * QW]
                        )
                    else:
                        nc.scalar.activation(pe[:], st[:], AF.Exp, scale=SCALE)
                    return pe

                pes = {}
                for pre in range(min(2, NB)):
                    pes[pre] = emit_sc_probs(pre)
                o_ps = l_ps = None
                for bi in range(NB):
                    qc, jj = blocks[bi]
                    if jj == 0:
                        o_ps = [
                            psp.tile([128, DV], F32, name=f"o_ps{i}", tag=f"o{i}")
                            for i in range(NDV)
                        ]
                        l_ps = psp.tile([128, 1], F32, name="l_ps", tag="l0")
                    if bi + 2 < NB:
                        pes[bi + 2] = emit_sc_probs(bi + 2)
                    pe = pes.pop(bi)
                    last = jj == C[qc] - 1
                    nc.tensor.matmul(
                        l_ps[:],
                        lhsT=pe[:],
                        rhs=ones[:],
                        start=(jj == 0),
                        stop=last,
                    )
                    for dvc in range(NDV):
                        nc.tensor.matmul(
                            o_ps[dvc][:],
                            lhsT=pe[:],
                            rhs=v_sb[:, jj * D + dvc * DV: jj * D + dvc * DV + DV],
                            start=(jj == 0),
                            stop=last,
                        )
                    if last:
                        # Clamp l away from 0 so fully-masked rows yield
                        # 0 (matching the reference), not 0 * inf = NaN.
                        rc = rcp.tile([128, 1], F32, name="rc", tag="rc")
                        lc = rcp.tile([128, 1], F32, name="lc", tag="lc")
                        nc.vector.tensor_scalar_max(lc[:], l_ps[:], 1e-30)
                        nc.vector.reciprocal(rc[:], lc[:])
                        ot = otp.tile([128, D], BF16, name="ot", tag="ot")
                        nc.vector.tensor_scalar_mul(ot[:, :DV], o_ps[0][:], rc[:])
                        nc.scalar.activation(
                            ot[:, DV:], o_ps[1][:], AF.Copy, scale=rc[:],
                        )
                        nc.sync.dma_start(
                            out[qc * QW: qc * QW + QW, :], ot[:]
                        )
    _elide_transitive_waits(nc)
    return nc


def _elide_transitive_waits(nc):
    """Drop semaphore waits already implied transitively.

    Hardware matmul (fused LDWEIGHTS) and DMA instruction encodings accept
    only ONE sync wait.  Tile's wait assignment is per-proc minimal but NOT
    transitive, so phase boundaries emit multi-wait matmuls/DMAs.  This pass
    walks the scheduled program (list order is a valid linearization),
    maintains a transitive vector clock per proc (engines and DMA queues are
    each FIFO), and removes waits that are (a) on the instruction's own proc
    (FIFO completion order), or (b) already implied by an earlier retained
    wait's transitive closure.
    """
    import re
    _proc_re = re.compile(r"^(PE|DVE|ACT|Act|Activation|SP|Pool|POOL|DMAHW\d+|DMASW\d+)_")

    def _is_proc_sem(name):
        return bool(_proc_re.match(name or ""))

    hist = {}      # sem id -> list of (tick, snapshot dict)
    state = {}     # proc key -> dict(sem id -> observed tick)
    tickc = {}     # sem id -> cumulative tick

    def snap_at(sem, t):
        h = hist.get(sem)
        if not h:
            return None
        lo, hi, best = 0, len(h) - 1, None
        while lo <= hi:
            mid = (lo + hi) // 2
            if h[mid][0] <= t:
                best = h[mid][1]
                lo = mid + 1
            else:
                hi = mid - 1
        return best

    splits = []
    for blk in nc.m.functions[0].blocks:
        for idx, i in enumerate(blk.instructions):
            si = i.sync_info
            if si is None:
                continue
            ups = [u for u in si.on_update if _is_proc_sem(u.ant_name)]
            own = ups[0].id if ups else ("eng", str(i.engine))
            v = state.setdefault(own, {})
            keep = []
            for w in list(si.on_wait):
                if (
                    w.wait_mode != "sem-ge-imm"
                    or w.wait_reg is not None
                    or not _is_proc_sem(w.ant_name)
                ):
                    keep.append(w)
                    continue
                # Same-proc elision is ONLY safe for PE matmuls: the PE
                # completes matmuls strictly in order (pc-monotone ends), so
                # a PE-self completion wait is redundant.  Other engines have
                # deep pipelines where same-engine WAR/WAW needs the wait.
                pe_self = (
                    w.id == own
                    and type(i).__name__ in ("InstMatmult", "InstLdweights")
                    and w.ant_name.startswith("PE")
                )
                if pe_self or v.get(w.id, 0) >= w.wait_value:
                    continue  # implied: PE FIFO or transitive closure
                keep.append(w)
                v[w.id] = max(v.get(w.id, 0), w.wait_value)
                s = snap_at(w.id, w.wait_value)
                if s:
                    for k2, t2 in s.items():
                        if v.get(k2, 0) < t2:
                            v[k2] = t2
            if len(keep) > 1 and all(_is_proc_sem(w.ant_name) for w in keep):
                # Hardware instruction encodings here accept at most one
                # sync wait: hoist all waits onto standalone sequencer
                # event-semaphore wait ops inserted just before.
                for k, w in enumerate(keep):
                    splits.append(
                        (blk, idx, mybir.InstEventSemaphore(
                            name=f"{i.name}-w{k}",
                            engine=i.engine,
                            sync_info=mybir.SyncInfo(on_wait=[w], on_update=[]),
                        ))
                    )
                keep = []
            if len(keep) != len(si.on_wait):
                si.on_wait = keep
                i.sync_info = si
            for u in ups:
                inc = u.update_value if u.update_mode in ("sem-inc", "sem-add-imm") else 0
                t = tickc.get(u.id, 0) + (inc or 0)
                tickc[u.id] = t
                snapshot = dict(v)
                snapshot[u.id] = t
                hist.setdefault(u.id, []).append((t, snapshot))
            nm = type(i).__name__
            if nm in ("InstMatmult", "InstDMACopy", "InstTensorCopy",
                      "InstTensorTensor", "InstActivation", "InstMemset",
                      "InstTensorScalarPtr", "InstReciprocal", "InstLdweights"):
                assert len(i.sync_info.on_wait) <= 1, (
                    i.name, nm,
                    [(w.ant_name, w.wait_value) for w in i.sync_info.on_wait],
                )
    by_blk = {}
    for blk, idx, inst in splits:
        by_blk.setdefault(id(blk), (blk, []))[1].append((idx, inst))
    for blk, items in by_blk.values():
        for idx, inst in sorted(items, key=lambda t: -t[0]):
            nc.register_instruction(inst)
            blk.instructions.insert(idx, inst)


_CACHE = {}


def _get_nc(structure=None):
    if structure is None:
        structure = _CACHE.get("struct", _CAUSAL)
    key = ("nc", structure)
    if key not in _CACHE:
        _CACHE[key] = _build_nc(structure)
    return _CACHE[key]


def make_in_maps(x, mask, Wq, Wk, Wv):
    x = np.asarray(x, dtype=np.float32)
    mask = np.asarray(mask)
    structure = _mask_structure(mask)
    _CACHE["struct"] = structure
    mix = _mix_order(structure)
    Wq = np.asarray(Wq, np.float32)
    Wk = np.asarray(Wk, np.float32)
    Wv = np.asarray(Wv, np.float32)
    # Weight layouts (shared by all cores), bf16:
    #   wq3/wk3[dc, p, mc*128+c] = W[dc*128+c, mc*128+p]
    #   wv3[dvc, p, mc*DV+c]     = Wv[dvc*DV+c, mc*128+p]
    wq3 = np.ascontiguousarray(
        Wq.reshape(NDC, 128, NMC, 128).transpose(0, 3, 2, 1).reshape(NDC, 128, NMC * 128)
    ).astype(BF)
    wk3 = np.ascontiguousarray(
        Wk.reshape(NDC, 128, NMC, 128).transpose(0, 3, 2, 1).reshape(NDC, 128, NMC * 128)
    ).astype(BF)
    wv3 = np.ascontiguousarray(
        Wv.reshape(NDV, DV, NMC, 128).transpose(0, 3, 2, 1).reshape(NDV, 128, NMC * DV)
    ).astype(BF)
    # xkv3[kc, p, mc*256+c] = x[b, kc*256+c, mc*128+p]  (per batch)
    xkv_b = {}
    for b in range(B):
        xkv_b[b] = np.ascontiguousarray(
            x[b].reshape(SKC, 256, NMC, 128).transpose(0, 3, 2, 1).reshape(SKC, 128, NMC * 256)
        ).astype(BF)
    in_maps = []
    for c in range(8):
        b, h = divmod(c, 2)
        rows = _rows(h)
        # xq3[mc, p, q] = x[b, rows[q], mc*128+p]
        xq3 = np.ascontiguousarray(
            x[b][rows].T.reshape(NMC, 128, NQ)
        ).astype(BF)
        mb = mask[b][rows]  # [1024 q, 2048 k]
        if mix:
            mmh = np.concatenate(
                [
                    mb[qc * QW:(qc + 1) * QW, j * 128:(j + 1) * 128].T
                    for (qc, j) in mix
                ],
                axis=1,
            ).astype(BF)
        else:
            mmh = np.zeros((128, QW), BF)
        in_maps.append(
            dict(
                xq3=xq3,
                xkv3=xkv_b[b],
                wq3=wq3,
                wk3=wk3,
                wv3=wv3,
                mmh=np.ascontiguousarray(mmh),
            )
        )
    return in_maps


def assemble(results):
    out = np.empty((B, S, D), np.float32)
    for c in range(8):
        b, h = divmod(c, 2)
        out[b, _rows(h)] = results[c]["out"]
    return out


def expected_core_out(expected, core):
    b, h = divmod(core, 2)
    return np.asarray(expected)[b][_rows(h)]


def kernel(x, mask, Wq, Wk, Wv):
    in_maps = make_in_maps(x, mask, Wq, Wk, Wv)
    nc = _get_nc(_CACHE["struct"])
    res = run_bass_kernel_spmd(nc, in_maps, list(range(8)))
    return assemble(res.results)


# revision 17
# speedup vs baseline: 1.1686x; 1.1686x over previous
"""Causal attention (B=4, S=2048, D=1024) on 8 Trainium2 NeuronCores.

Sharding: core c = (batch b = c//2, parity h = c%2). Each core owns the
1024 query rows of batch b with (q // 64) % 2 == h, sorted ascending
(64-row interleave). This makes the causal work profile identical on
every core: local query chunk qc (128 rows) spans global 128-row classes
2qc and 2qc+1, so it needs exactly the key-block prefix 0..2qc+1. All
cores run the same SPMD program with per-chunk key-block counts
C = [2,4,...,16] (72 score blocks/core instead of 128 unmasked), and the
per-batch work is exactly balanced between the two cores.

The program is specialized on the observed mask block structure
(computed on host in make_in_maps, cached per structure): fully-kept
score blocks skip masking entirely; partially-masked blocks multiply
exp(scores) by a {0,1} bf16 mask tile (SBUF-resident, 1MB/core vs the
33MB fp32 additive-bias tensor this replaces). Arbitrary masks fall
back to C=[16,16,16,16] with per-block masking - always correct, just
slower.

All inputs are converted to bf16 and pre-shuffled on the host into the
exact SBUF layout each consumer reads, so every load is one contiguous
[128, N] DMA with no on-chip staging copies, and all three weight
matrices fit in SBUF simultaneously and are prefetched up front (the
phase-boundary stalls of the staged variant came from weight loads
having to wait for the previous phase's SBUF space to free).

Per-core kernel (SPMD, identical program, per-core data):
  Phase A: fused K^T/V projection streaming x^T key-chunks (V one chunk
           behind K so the V weights' arrival is off the critical path),
           then Q^T. All outputs bf16, SBUF-resident.
  Phase B: transposed-scores flash attention, software-pipelined so the
           PE never waits on the exp/mask chain: score matmuls for block
           i+2 are issued between the probability consumption (l/O
           matmuls) of blocks i-1 and i. S^T = K @ Q^T, exp on ScalarE
           (no max subtraction; scores are ~N(0,1) by construction),
           P in bf16, O = P @ V and l = P^T-column sums accumulated in
           PSUM across the block prefix, then normalize O by 1/l
           (split across DVE and ACT) and DMA out.
"""

import sys

sys.path.insert(0, "/opt/trn_rl_repo")

import numpy as np
import ml_dtypes

import concourse.bass as bass
import concourse.mybir as mybir
from concourse import tile
from concourse.bass_utils import run_bass_kernel_spmd

F32 = mybir.dt.float32
BF16 = mybir.dt.bfloat16
AF = mybir.ActivationFunctionType
BF = ml_dtypes.bfloat16

B, S, D = 4, 2048, 1024
NQ = 1024          # query rows per core
NKB = 16           # key blocks of 128
NQC = 8            # query column chunks per core
QW = 128           # query width of one score tile
NMC = 8            # d_model chunks of 128 (contraction)
NDC = 8            # d_k chunks of 128
SKC = 8            # streamed key chunks of 256 in phase A
DV = 512           # v-column tile width
NDV = D // DV
NQB = QW // 128
SCALE = 1.0 / 32.0  # 1/sqrt(D_K)

# Local row l of core parity h <-> global query row (l//64)*128 + h*64 + l%64.
_LROWS = (np.arange(NQ) // 64) * 128 + (np.arange(NQ) % 64)


def _rows(h):
    return _LROWS + h * 64


# Causal structure: chunk qc (128 rows = classes 2qc,2qc+1) needs key
# blocks 0..2qc+1; the two diagonal-straddling blocks are partially masked.
_CAUSAL = (
    tuple(2 * qc + 2 for qc in range(NQC)),
    frozenset((qc, j) for qc in range(NQC) for j in range(2 * qc, 2 * qc + 2)),
)


def _mask_structure(mask):
    """(C, mixed) uniform across all 8 cores for the observed mask.

    C[qc]: number of key blocks (prefix 0..C-1) chunk qc computes.
    mixed: (qc, j) blocks that are not all-True on every core and thus
    get an explicit multiplicative mask tile.
    Requires every needed key block to sit in a prefix; C=16 everywhere
    is the always-valid fallback.
    """
    alls = np.ones((NQC, NKB), bool)
    anys = np.zeros((NQC, NKB), bool)
    for b in range(B):
        for h in range(2):
            m = mask[b][_rows(h)]  # [1024, 2048]
            mr = m.reshape(NQC, QW, NKB, 128)
            alls &= mr.all(axis=(1, 3))
            anys |= mr.any(axis=(1, 3))
    C = []
    for qc in range(NQC):
        need = np.nonzero(anys[qc])[0]
        C.append(int(need[-1]) + 1 if len(need) else 1)
    mixed = frozenset(
        (qc, j) for qc in range(NQC) for j in range(C[qc]) if not alls[qc, j]
    )
    return (tuple(C), mixed)


def _mix_order(structure):
    C, mixed = structure
    return [(qc, j) for qc in range(NQC) for j in range(C[qc]) if (qc, j) in mixed]


def _build_nc(structure):
    C, mixed = structure
    nmix = max(1, len(mixed))
    mix_index = {qj: i for i, qj in enumerate(_mix_order(structure))}

    nc = bass.Bass()
    # All inputs bf16, host-pre-shuffled into exact SBUF layouts.
    xq3 = nc.declare_dram_parameter("xq3", [NMC, 128, NQ], BF16, isOutput=False)
    xkv3 = nc.declare_dram_parameter("xkv3", [SKC, 128, NMC * 256], BF16, isOutput=False)
    wq3 = nc.declare_dram_parameter("wq3", [NDC, 128, NMC * 128], BF16, isOutput=False)
    wk3 = nc.declare_dram_parameter("wk3", [NDC, 128, NMC * 128], BF16, isOutput=False)
    wv3 = nc.declare_dram_parameter("wv3", [NDV, 128, NMC * DV], BF16, isOutput=False)
    mmh = nc.declare_dram_parameter("mmh", [128, nmix * QW], BF16, isOutput=False)
    out = nc.declare_dram_parameter("out", [NQ, D], BF16, isOutput=True)

    with tile.TileContext(nc) as tc:
        with tc.tile_pool(name="res", bufs=1) as res, \
             tc.tile_pool(name="xcp", bufs=2) as xcp, \
             tc.tile_pool(name="psp", bufs=1, space="PSUM") as psp:
            # Resident: Q^T [p=dk, dc, q]; K^T [p=dk, dc, k]; V [p=k, kb, dv].
            qt_sb = res.tile([128, NDC * NQ], BF16, name="qt_sb")
            kt_sb = res.tile([128, NDC * S], BF16, name="kt_sb")
            v_sb = res.tile([128, NKB * D], BF16, name="v_sb")
            wq_sb = res.tile([128, NDC * NMC * 128], BF16, name="wq_sb")
            wk_sb = res.tile([128, NDC * NMC * 128], BF16, name="wk_sb")
            wv_sb = res.tile([128, NDV * NMC * DV], BF16, name="wv_sb")
            xq_sb = res.tile([128, NMC * NQ], BF16, name="xq_sb")
            mm_sb = res.tile([128, nmix * QW], BF16, name="mm_sb")
            ones = res.tile([128, 1], BF16, name="ones")
            nc.vector.memset(ones[:], 1.0)
            scr = res.tile([128, 512], BF16, name="scr")
            nc.vector.memset(scr[:], 0.0)

            # DMA issue order == need order: the cost of every transfer is
            # serialized on the global DMA engines, so first-needed-first.
            xcols = {}

            def fetch_xcol(kc):
                xcol = xcp.tile([128, NMC * 256], BF16, name="xcol", tag="xcol", bufs=3)
                nc.sync.dma_start(xcol[:], xkv3[kc])
                xcols[kc] = xcol

            nc.sync.dma_start(wk_sb[:, 0:1024], wk3[0])
            fetch_xcol(0)
            nc.sync.dma_start(wk_sb[:, 1024:2048], wk3[1])
            nc.sync.dma_start(wk_sb[:, 2048:3072], wk3[2])
            fetch_xcol(1)
            for dc in range(3, NDC):
                nc.sync.dma_start(wk_sb[:, dc * 1024:(dc + 1) * 1024], wk3[dc])
            for dvc in range(NDV):
                nc.sync.dma_start(wv_sb[:, dvc * 4096:(dvc + 1) * 4096], wv3[dvc])

            # PE warm-up: the tensor engine only reaches full clock after
            # ~3us of continuous execution. Filler matmuls on zeroed scratch
            # bridge the initial input-DMA latency so the first real
            # projection matmuls run at full rate instead of half.
            for f in range(10):
                ps = psp.tile([128, 512], F32, name=f"fill{f}", tag="st", bufs=3)
                nc.tensor.matmul(
                    ps[:], lhsT=scr[:, :128], rhs=scr[:], start=True, stop=True
                )

            # ---- Phase A: fused K^T / V projection (V one chunk behind K),
            # then Q^T. K^T = Wk @ x^T; V = x @ Wv^T; Q^T = Wq @ x_q^T.
            for kc in range(SKC + 1):
                if kc < SKC:
                    xcol = xcols[kc]
                    for dc in range(NDC):
                        ps = psp.tile([128, 512], F32, name="psk", tag="st", bufs=3)[:, :256]
                        for mc in range(NMC):
                            nc.tensor.matmul(
                                ps[:],
                                lhsT=wk_sb[:, dc * 1024 + mc * 128: dc * 1024 + mc * 128 + 128],
                                rhs=xcol[:, mc * 256:(mc + 1) * 256],
                                start=(mc == 0),
                                stop=(mc == NMC - 1),
                            )
                        nc.vector.tensor_copy(
                            kt_sb[:, dc * S + kc * 256: dc * S + kc * 256 + 256], ps[:]
                        )
                if kc >= 1:
                    vcol = xcols.pop(kc - 1)
                    for kbl in range(2):
                        kb = (kc - 1) * 2 + kbl
                        for dvc in range(NDV):
                            ps = psp.tile([128, 512], F32, name="psv", tag="st", bufs=3)[:, :DV]
                            for mc in range(NMC):
                                nc.tensor.matmul(
                                    ps[:],
                                    lhsT=vcol[:, mc * 256 + kbl * 128: mc * 256 + kbl * 128 + 128],
                                    rhs=wv_sb[:, dvc * 4096 + mc * DV: dvc * 4096 + mc * DV + DV],
                                    start=(mc == 0),
                                    stop=(mc == NMC - 1),
                                )
                            nc.scalar.activation(
                                v_sb[:, kb * D + dvc * DV: kb * D + dvc * DV + DV],
                                ps[:], AF.Copy,
                            )
                if kc + 2 < SKC:
                    fetch_xcol(kc + 2)
            # Late prefetches: issued behind all x-column traffic, land long
            # before phase A ends.
            for mc in range(NMC):
                nc.sync.dma_start(xq_sb[:, mc * NQ:(mc + 1) * NQ], xq3[mc])
            for dc in range(NDC):
                nc.sync.dma_start(wq_sb[:, dc * 1024:(dc + 1) * 1024], wq3[dc])
            nc.sync.dma_start(mm_sb[:], mmh[:])
            for dc in range(NDC):
                for q2 in range(NQ // 512):
                    ps = psp.tile([128, 512], F32, name="psq", tag="st", bufs=3)
                    for mc in range(NMC):
                        nc.tensor.matmul(
                            ps[:],
                            lhsT=wq_sb[:, dc * 1024 + mc * 128: dc * 1024 + mc * 128 + 128],
                            rhs=xq_sb[:, mc * NQ + q2 * 512: mc * NQ + q2 * 512 + 512],
                            start=(mc == 0),
                            stop=(mc == NMC - 1),
                        )
                    nc.vector.tensor_copy(
                        qt_sb[:, dc * NQ + q2 * 512: dc * NQ + q2 * 512 + 512], ps[:]
                    )

            # ---------------- Phase B: attention ----------------
            with (
                tc.tile_pool(name="pap", bufs=3) as pap,
                tc.tile_pool(name="pep", bufs=3) as pep,
                tc.tile_pool(name="otp", bufs=4) as otp,
                tc.tile_pool(name="rcp", bufs=2) as rcp,
            ):
                blocks = [(qc, j) for qc in range(NQC) for j in range(C[qc])]
                NB = len(blocks)

                def emit_sc_probs(bi):
                    # Score matmuls + exp (+ mask) for block bi; returns the
                    # bf16 probability tile consumed by the l/O matmuls.
                    qc, j = blocks[bi]
                    st = psp.tile([128, 512], F32, name="st", tag="st", bufs=3)[:, :QW]
                    for dc in range(NDC):
                        nc.tensor.matmul(
                            st[:],
                            lhsT=kt_sb[:, dc * S + j * 128: dc * S + j * 128 + 128],
                            rhs=qt_sb[:, dc * NQ + qc * QW: dc * NQ + qc * QW + QW],
                            start=(dc == 0),
                            stop=(dc == NDC - 1),
                        )
                    pe = pep.tile([128, QW], BF16, name="pe", tag="pe")
                    if (qc, j) in mix_index:
                        mi = mix_index[(qc, j)]
                        pa = pap.tile([128, QW], BF16, name="pa", tag="pa")
                        nc.scalar.activation(pa[:], st[:], AF.Exp, scale=SCALE)
                        nc.vector.tensor_mul(
                            pe[:], pa[:], mm_sb[:, mi * QW:(mi + 1) * QW]
                        )
                    else:
                        nc.scalar.activation(pe[:], st[:], AF.Exp, scale=SCALE)
                    return pe

                pes = {}
                for pre in range(min(2, NB)):
                    pes[pre] = emit_sc_probs(pre)
                o_ps = l_ps = None
                for bi in range(NB):
                    qc, jj = blocks[bi]
                    if jj == 0:
                        o_ps = [
                            psp.tile([128, DV], F32, name=f"o_ps{i}", tag=f"o{i}")
                            for i in range(NDV)
                        ]
                        l_ps = psp.tile([128, 1], F32, name="l_ps", tag="l0")
                    if bi + 2 < NB:
                        pes[bi + 2] = emit_sc_probs(bi + 2)
                    pe = pes.pop(bi)
                    last = jj == C[qc] - 1
                    nc.tensor.matmul(
                        l_ps[:],
                        lhsT=pe[:],
                        rhs=ones[:],
                        start=(jj == 0),
                        stop=last,
                    )
                    for dvc in range(NDV):
                        nc.tensor.matmul(
                            o_ps[dvc][:],
                            lhsT=pe[:],
                            rhs=v_sb[:, jj * D + dvc * DV: jj * D + dvc * DV + DV],
                            start=(jj == 0),
                            stop=last,
                        )
                    if last:
                        # Clamp l away from 0 so fully-masked rows yield
                        # 0 (matching the reference), not 0 * inf = NaN.
                        rc = rcp.tile([128, 1], F32, name="rc", tag="rc")
                        lc = rcp.tile([128, 1], F32, name="lc", tag="lc")
                        nc.vector.tensor_scalar_max(lc[:], l_ps[:], 1e-30)
                        nc.vector.reciprocal(rc[:], lc[:])
                        ot = otp.tile([128, D], BF16, name="ot", tag="ot")
                        nc.vector.tensor_scalar_mul(ot[:, :DV], o_ps[0][:], rc[:])
                        nc.scalar.activation(
                            ot[:, DV:], o_ps[1][:], AF.Copy, scale=rc[:],
                        )
                        nc.sync.dma_start(
                            out[qc * QW: qc * QW + QW, :], ot[:]
                        )
    _elide_transitive_waits(nc)
    return nc


def _elide_transitive_waits(nc):
    """Drop semaphore waits already implied transitively.

    Hardware matmul (fused LDWEIGHTS) and DMA instruction encodings accept
    only ONE sync wait.  Tile's wait assignment is per-proc minimal but NOT
    transitive, so phase boundaries emit multi-wait matmuls/DMAs.  This pass
    walks the scheduled program (list order is a valid linearization),
    maintains a transitive vector clock per proc (engines and DMA queues are
    each FIFO), and removes waits that are (a) on the instruction's own proc
    (FIFO completion order), or (b) already implied by an earlier retained
    wait's transitive closure.
    """
    import re
    _proc_re = re.compile(r"^(PE|DVE|ACT|Act|Activation|SP|Pool|POOL|DMAHW\d+|DMASW\d+)_")

    def _is_proc_sem(name):
        return bool(_proc_re.match(name or ""))

    hist = {}      # sem id -> list of (tick, snapshot dict)
    state = {}     # proc key -> dict(sem id -> observed tick)
    tickc = {}     # sem id -> cumulative tick

    def snap_at(sem, t):
        h = hist.get(sem)
        if not h:
            return None
        lo, hi, best = 0, len(h) - 1, None
        while lo <= hi:
            mid = (lo + hi) // 2
            if h[mid][0] <= t:
                best = h[mid][1]
                lo = mid + 1
            else:
                hi = mid - 1
        return best

    splits = []
    for blk in nc.m.functions[0].blocks:
        for idx, i in enumerate(blk.instructions):
            si = i.sync_info
            if si is None:
                continue
            ups = [u for u in si.on_update if _is_proc_sem(u.ant_name)]
            own = ups[0].id if ups else ("eng", str(i.engine))
            v = state.setdefault(own, {})
            keep = []
            for w in list(si.on_wait):
                if (
                    w.wait_mode != "sem-ge-imm"
                    or w.wait_reg is not None
                    or not _is_proc_sem(w.ant_name)
                ):
                    keep.append(w)
                    continue
                # Same-proc elision is ONLY safe for PE matmuls: the PE
                # completes matmuls strictly in order (pc-monotone ends), so
                # a PE-self completion wait is redundant.  Other engines have
                # deep pipelines where same-engine WAR/WAW needs the wait.
                pe_self = (
                    w.id == own
                    and type(i).__name__ in ("InstMatmult", "InstLdweights")
                    and w.ant_name.startswith("PE")
                )
                if pe_self or v.get(w.id, 0) >= w.wait_value:
                    continue  # implied: PE FIFO or transitive closure
                keep.append(w)
                v[w.id] = max(v.get(w.id, 0), w.wait_value)
                s = snap_at(w.id, w.wait_value)
                if s:
                    for k2, t2 in s.items():
                        if v.get(k2, 0) < t2:
                            v[k2] = t2
            if len(keep) > 1 and all(_is_proc_sem(w.ant_name) for w in keep):
                # Hardware instruction encodings here accept at most one
                # sync wait: hoist all waits onto standalone sequencer
                # event-semaphore wait ops inserted just before.
                for k, w in enumerate(keep):
                    splits.append(
                        (blk, idx, mybir.InstEventSemaphore(
                            name=f"{i.name}-w{k}",
                            engine=i.engine,
                            sync_info=mybir.SyncInfo(on_wait=[w], on_update=[]),
                        ))
                    )
                keep = []
            if len(keep) != len(si.on_wait):
                si.on_wait = keep
                i.sync_info = si
            for u in ups:
                inc = u.update_value if u.update_mode in ("sem-inc", "sem-add-imm") else 0
                t = tickc.get(u.id, 0) + (inc or 0)
                tickc[u.id] = t
                snapshot = dict(v)
                snapshot[u.id] = t
                hist.setdefault(u.id, []).append((t, snapshot))
            nm = type(i).__name__
            if nm in ("InstMatmult", "InstDMACopy", "InstTensorCopy",
                      "InstTensorTensor", "InstActivation", "InstMemset",
                      "InstTensorScalarPtr", "InstReciprocal", "InstLdweights"):
                assert len(i.sync_info.on_wait) <= 1, (
                    i.name, nm,
                    [(w.ant_name, w.wait_value) for w in i.sync_info.on_wait],
                )
    by_blk = {}
    for blk, idx, inst in splits:
        by_blk.setdefault(id(blk), (blk, []))[1].append((idx, inst))
    for blk, items in by_blk.values():
        for idx, inst in sorted(items, key=lambda t: -t[0]):
            nc.register_instruction(inst)
            blk.instructions.insert(idx, inst)


_CACHE = {}


def _get_nc(structure=None):
    if structure is None:
        structure = _CACHE.get("struct", _CAUSAL)
    key = ("nc", structure)
    if key not in _CACHE:
        _CACHE[key] = _build_nc(structure)
    return _CACHE[key]


def make_in_maps(x, mask, Wq, Wk, Wv):
    x = np.asarray(x, dtype=np.float32)
    mask = np.asarray(mask)
    structure = _mask_structure(mask)
    _CACHE["struct"] = structure
    mix = _mix_order(structure)
    Wq = np.asarray(Wq, np.float32)
    Wk = np.asarray(Wk, np.float32)
    Wv = np.asarray(Wv, np.float32)
    # Weight layouts (shared by all cores), bf16:
    #   wq3/wk3[dc, p, mc*128+c] = W[dc*128+c, mc*128+p]
    #   wv3[dvc, p, mc*DV+c]     = Wv[dvc*DV+c, mc*128+p]
    wq3 = np.ascontiguousarray(
        Wq.reshape(NDC, 128, NMC, 128).transpose(0, 3, 2, 1).reshape(NDC, 128, NMC * 128)
    ).astype(BF)
    wk3 = np.ascontiguousarray(
        Wk.reshape(NDC, 128, NMC, 128).transpose(0, 3, 2, 1).reshape(NDC, 128, NMC * 128)
    ).astype(BF)
    wv3 = np.ascontiguousarray(
        Wv.reshape(NDV, DV, NMC, 128).transpose(0, 3, 2, 1).reshape(NDV, 128, NMC * DV)
    ).astype(BF)
    # xkv3[kc, p, mc*256+c] = x[b, kc*256+c, mc*128+p]  (per batch)
    xkv_b = {}
    for b in range(B):
        xkv_b[b] = np.ascontiguousarray(
            x[b].reshape(SKC, 256, NMC, 128).transpose(0, 3, 2, 1).reshape(SKC, 128, NMC * 256)
        ).astype(BF)
    in_maps = []
    for c in range(8):
        b, h = divmod(c, 2)
        rows = _rows(h)
        # xq3[mc, p, q] = x[b, rows[q], mc*128+p]
        xq3 = np.ascontiguousarray(
            x[b][rows].T.reshape(NMC, 128, NQ)
        ).astype(BF)
        mb = mask[b][rows]  # [1024 q, 2048 k]
        if mix:
            mmh = np.concatenate(
                [
                    mb[qc * QW:(qc + 1) * QW, j * 128:(j + 1) * 128].T
                    for (qc, j) in mix
                ],
                axis=1,
            ).astype(BF)
        else:
            mmh = np.zeros((128, QW), BF)
        in_maps.append(
            dict(
                xq3=xq3,
                xkv3=xkv_b[b],
                wq3=wq3,
                wk3=wk3,
                wv3=wv3,
                mmh=np.ascontiguousarray(mmh),
            )
        )
    return in_maps


def assemble(results):
    out = np.empty((B, S, D), np.float32)
    for c in range(8):
        b, h = divmod(c, 2)
        out[b, _rows(h)] = results[c]["out"]
    return out


def expected_core_out(expected, core):
    b, h = divmod(core, 2)
    return np.asarray(expected)[b][_rows(h)]


def kernel(x, mask, Wq, Wk, Wv):
    in_maps = make_in_maps(x, mask, Wq, Wk, Wv)
    nc = _get_nc(_CACHE["struct"])
    res = run_bass_kernel_spmd(nc, in_maps, list(range(8)))
    return assemble(res.results)


# revision 19
# speedup vs baseline: 1.1789x; 1.0088x over previous
"""Causal attention (B=4, S=2048, D=1024) on 8 Trainium2 NeuronCores.

Sharding: core c = (batch b = c//2, parity h = c%2). Each core owns the
1024 query rows of batch b with (q // 64) % 2 == h, sorted ascending
(64-row interleave). This makes the causal work profile identical on
every core: local query chunk qc (128 rows) spans global 128-row classes
2qc and 2qc+1, so it needs exactly the key-block prefix 0..2qc+1. All
cores run the same SPMD program with per-chunk key-block counts
C = [2,4,...,16] (72 score blocks/core instead of 128 unmasked), and the
per-batch work is exactly balanced between the two cores.

The program is specialized on the observed mask block structure
(computed on host in make_in_maps, cached per structure): fully-kept
score blocks skip masking entirely; partially-masked blocks multiply
exp(scores) by a {0,1} bf16 mask tile (SBUF-resident, 1MB/core vs the
33MB fp32 additive-bias tensor this replaces). Arbitrary masks fall
back to C=[16,16,16,16] with per-block masking - always correct, just
slower.

All inputs are converted to bf16 and pre-shuffled on the host into the
exact SBUF layout each consumer reads, so every load is one contiguous
[128, N] DMA with no on-chip staging copies, and all three weight
matrices fit in SBUF simultaneously and are prefetched up front (the
phase-boundary stalls of the staged variant came from weight loads
having to wait for the previous phase's SBUF space to free).

Per-core kernel (SPMD, identical program, per-core data):
  Phase A: fused K^T/V projection streaming x^T key-chunks (V one chunk
           behind K so the V weights' arrival is off the critical path),
           then Q^T. All outputs bf16, SBUF-resident.
  Phase B: transposed-scores flash attention, software-pipelined so the
           PE never waits on the exp/mask chain: score matmuls for block
           i+2 are issued between the probability consumption (l/O
           matmuls) of blocks i-1 and i. S^T = K @ Q^T, exp on ScalarE
           (no max subtraction; scores are ~N(0,1) by construction),
           P in bf16, O = P @ V and l = P^T-column sums accumulated in
           PSUM across the block prefix, then normalize O by 1/l
           (split across DVE and ACT) and DMA out.
"""

import sys

sys.path.insert(0, "/opt/trn_rl_repo")

import numpy as np
import ml_dtypes

import concourse.bass as bass
import concourse.mybir as mybir
from concourse import tile
from concourse.bass_utils import run_bass_kernel_spmd

F32 = mybir.dt.float32
BF16 = mybir.dt.bfloat16
AF = mybir.ActivationFunctionType
BF = ml_dtypes.bfloat16

B, S, D = 4, 2048, 1024
NQ = 1024          # query rows per core
NKB = 16           # key blocks of 128
NQC = 8            # query column chunks per core
QW = 128           # query width of one score tile
NMC = 8            # d_model chunks of 128 (contraction)
NDC = 8            # d_k chunks of 128
SKC = 8            # streamed key chunks of 256 in phase A
DV = 512           # v-column tile width
NDV = D // DV
NQB = QW // 128
SCALE = 1.0 / 32.0  # 1/sqrt(D_K)

# Local row l of core parity h <-> global query row (l//64)*128 + h*64 + l%64.
_LROWS = (np.arange(NQ) // 64) * 128 + (np.arange(NQ) % 64)


def _rows(h):
    return _LROWS + h * 64


# Causal structure: chunk qc (128 rows = classes 2qc,2qc+1) needs key
# blocks 0..2qc+1; the two diagonal-straddling blocks are partially masked.
_CAUSAL = (
    tuple(2 * qc + 2 for qc in range(NQC)),
    frozenset((qc, j) for qc in range(NQC) for j in range(2 * qc, 2 * qc + 2)),
)


def _mask_structure(mask):
    """(C, mixed) uniform across all 8 cores for the observed mask.

    C[qc]: number of key blocks (prefix 0..C-1) chunk qc computes.
    mixed: (qc, j) blocks that are not all-True on every core and thus
    get an explicit multiplicative mask tile.
    Requires every needed key block to sit in a prefix; C=16 everywhere
    is the always-valid fallback.
    """
    alls = np.ones((NQC, NKB), bool)
    anys = np.zeros((NQC, NKB), bool)
    for b in range(B):
        for h in range(2):
            m = mask[b][_rows(h)]  # [1024, 2048]
            mr = m.reshape(NQC, QW, NKB, 128)
            alls &= mr.all(axis=(1, 3))
            anys |= mr.any(axis=(1, 3))
    C = []
    for qc in range(NQC):
        need = np.nonzero(anys[qc])[0]
        C.append(int(need[-1]) + 1 if len(need) else 1)
    mixed = frozenset(
        (qc, j) for qc in range(NQC) for j in range(C[qc]) if not alls[qc, j]
    )
    return (tuple(C), mixed)


def _mix_order(structure):
    C, mixed = structure
    return [(qc, j) for qc in range(NQC) for j in range(C[qc]) if (qc, j) in mixed]


def _build_nc(structure):
    C, mixed = structure
    nmix = max(1, len(mixed))
    mix_index = {qj: i for i, qj in enumerate(_mix_order(structure))}

    nc = bass.Bass()
    # All inputs bf16, host-pre-shuffled into exact SBUF layouts.
    xq3 = nc.declare_dram_parameter("xq3", [NMC, 128, NQ], BF16, isOutput=False)
    xkv3 = nc.declare_dram_parameter("xkv3", [SKC, 128, NMC * 256], BF16, isOutput=False)
    wq3 = nc.declare_dram_parameter("wq3", [NDC, 128, NMC * 128], BF16, isOutput=False)
    wk3 = nc.declare_dram_parameter("wk3", [NDC, 128, NMC * 128], BF16, isOutput=False)
    wv3 = nc.declare_dram_parameter("wv3", [NDV, 128, NMC * DV], BF16, isOutput=False)
    mmh = nc.declare_dram_parameter("mmh", [128, nmix * QW], BF16, isOutput=False)
    out = nc.declare_dram_parameter("out", [NQ, D], BF16, isOutput=True)

    with tile.TileContext(nc) as tc:
        with tc.tile_pool(name="res", bufs=1) as res, \
             tc.tile_pool(name="xcp", bufs=2) as xcp, \
             tc.tile_pool(name="psp", bufs=1, space="PSUM") as psp:
            # Resident: Q^T [p=dk, dc, q]; K^T [p=dk, dc, k]; V [p=k, kb, dv].
            qt_sb = res.tile([128, NDC * NQ], BF16, name="qt_sb")
            kt_sb = res.tile([128, NDC * S], BF16, name="kt_sb")
            v_sb = res.tile([128, NKB * D], BF16, name="v_sb")
            wq_sb = res.tile([128, NDC * NMC * 128], BF16, name="wq_sb")
            wk_sb = res.tile([128, NDC * NMC * 128], BF16, name="wk_sb")
            wv_sb = res.tile([128, NDV * NMC * DV], BF16, name="wv_sb")
            xq_sb = res.tile([128, NMC * NQ], BF16, name="xq_sb")
            mm_sb = res.tile([128, nmix * QW], BF16, name="mm_sb")
            ones = res.tile([128, 1], BF16, name="ones")
            nc.vector.memset(ones[:], 1.0)
            scr = res.tile([128, 512], BF16, name="scr")
            nc.gpsimd.memset(scr[:], 0.0)

            # DMA issue order == need order: the cost of every transfer is
            # serialized on the global DMA engines, so first-needed-first.
            xcols = {}

            def fetch_xcol(kc):
                xcol = xcp.tile([128, NMC * 256], BF16, name="xcol", tag="xcol", bufs=3)
                nc.sync.dma_start(xcol[:], xkv3[kc])
                xcols[kc] = xcol

            nc.sync.dma_start(wk_sb[:, 0:1024], wk3[0])
            fetch_xcol(0)
            nc.sync.dma_start(wk_sb[:, 1024:2048], wk3[1])
            nc.sync.dma_start(wk_sb[:, 2048:3072], wk3[2])
            fetch_xcol(1)
            for dc in range(3, NDC):
                nc.sync.dma_start(wk_sb[:, dc * 1024:(dc + 1) * 1024], wk3[dc])
            for dvc in range(NDV):
                nc.sync.dma_start(wv_sb[:, dvc * 4096:(dvc + 1) * 4096], wv3[dvc])

            # PE warm-up: the tensor engine only reaches full clock after
            # ~3us of continuous execution. Filler matmuls on zeroed scratch
            # bridge the initial input-DMA latency so the first real
            # projection matmuls run at full rate instead of half.
            for f in range(10):
                ps = psp.tile([128, 512], F32, name=f"fill{f}", tag="st", bufs=3)
                nc.tensor.matmul(
                    ps[:], lhsT=scr[:, :128], rhs=scr[:], start=True, stop=True
                )

            # ---- Phase A: fused K^T / V projection (V one chunk behind K),
            # then Q^T. K^T = Wk @ x^T; V = x @ Wv^T; Q^T = Wq @ x_q^T.
            for kc in range(SKC + 1):
                if kc < SKC:
                    xcol = xcols[kc]
                    for dc in range(NDC):
                        ps = psp.tile([128, 512], F32, name="psk", tag="st", bufs=3)[:, :256]
                        for mc in range(NMC):
                            nc.tensor.matmul(
                                ps[:],
                                lhsT=wk_sb[:, dc * 1024 + mc * 128: dc * 1024 + mc * 128 + 128],
                                rhs=xcol[:, mc * 256:(mc + 1) * 256],
                                start=(mc == 0),
                                stop=(mc == NMC - 1),
                            )
                        nc.vector.tensor_copy(
                            kt_sb[:, dc * S + kc * 256: dc * S + kc * 256 + 256], ps[:]
                        )
                if kc >= 1:
                    vcol = xcols.pop(kc - 1)
                    for kbl in range(2):
                        kb = (kc - 1) * 2 + kbl
                        for dvc in range(NDV):
                            ps = psp.tile([128, 512], F32, name="psv", tag="st", bufs=3)[:, :DV]
                            for mc in range(NMC):
                                nc.tensor.matmul(
                                    ps[:],
                                    lhsT=vcol[:, mc * 256 + kbl * 128: mc * 256 + kbl * 128 + 128],
                                    rhs=wv_sb[:, dvc * 4096 + mc * DV: dvc * 4096 + mc * DV + DV],
                                    start=(mc == 0),
                                    stop=(mc == NMC - 1),
                                )
                            nc.scalar.activation(
                                v_sb[:, kb * D + dvc * DV: kb * D + dvc * DV + DV],
                                ps[:], AF.Copy,
                            )
                if kc + 2 < SKC:
                    fetch_xcol(kc + 2)
            # Late prefetches: issued behind all x-column traffic, land long
            # before phase A ends.
            for mc in range(NMC):
                nc.sync.dma_start(xq_sb[:, mc * NQ:(mc + 1) * NQ], xq3[mc])
            for dc in range(NDC):
                nc.sync.dma_start(wq_sb[:, dc * 1024:(dc + 1) * 1024], wq3[dc])
            nc.sync.dma_start(mm_sb[:], mmh[:])
            for dc in range(NDC):
                for q2 in range(NQ // 512):
                    ps = psp.tile([128, 512], F32, name="psq", tag="st", bufs=3)
                    for mc in range(NMC):
                        nc.tensor.matmul(
                            ps[:],
                            lhsT=wq_sb[:, dc * 1024 + mc * 128: dc * 1024 + mc * 128 + 128],
                            rhs=xq_sb[:, mc * NQ + q2 * 512: mc * NQ + q2 * 512 + 512],
                            start=(mc == 0),
                            stop=(mc == NMC - 1),
                        )
                    nc.vector.tensor_copy(
                        qt_sb[:, dc * NQ + q2 * 512: dc * NQ + q2 * 512 + 512], ps[:]
                    )

            # ---------------- Phase B: attention ----------------
            with (
                tc.tile_pool(name="pap", bufs=3) as pap,
                tc.tile_pool(name="pep", bufs=3) as pep,
                tc.tile_pool(name="otp", bufs=4) as otp,
                tc.tile_pool(name="rcp", bufs=2) as rcp,
            ):
                blocks = [(qc, j) for qc in range(NQC) for j in range(C[qc])]
                NB = len(blocks)

                # For the causal structure, the odd-diagonal block of each
                # chunk (j == 2qc+1) has its first 64 query columns fully
                # masked on both parities: those rows (class 2qc) precede
                # every key of block 2qc+1. Compute only the right half of
                # the scores and zero the left half of the probabilities.
                halves = (
                    {(qc, 2 * qc + 1) for qc in range(NQC)}
                    if structure == _CAUSAL
                    else set()
                )

                def emit_sc_probs(bi):
                    # Score matmuls + exp (+ mask) for block bi; returns the
                    # bf16 probability tile consumed by the l/O matmuls.
                    qc, j = blocks[bi]
                    lo = 64 if (qc, j) in halves else 0
                    st = psp.tile([128, 512], F32, name="st", tag="st", bufs=3)[:, :QW]
                    for dc in range(NDC):
                        nc.tensor.matmul(
                            st[:, lo:],
                            lhsT=kt_sb[:, dc * S + j * 128: dc * S + j * 128 + 128],
                            rhs=qt_sb[:, dc * NQ + qc * QW + lo: dc * NQ + qc * QW + QW],
                            start=(dc == 0),
                            stop=(dc == NDC - 1),
                        )
                    pe = pep.tile([128, QW], BF16, name="pe", tag="pe")
                    if lo:
                        nc.vector.memset(pe[:, :lo], 0.0)
                    if (qc, j) in mix_index:
                        mi = mix_index[(qc, j)]
                        pa = pap.tile([128, QW], BF16, name="pa", tag="pa")
                        nc.scalar.activation(pa[:, lo:], st[:, lo:], AF.Exp, scale=SCALE)
                        nc.vector.tensor_mul(
                            pe[:, lo:], pa[:, lo:],
                            mm_sb[:, mi * QW + lo:(mi + 1) * QW],
                        )
                    else:
                        nc.scalar.activation(pe[:, lo:], st[:, lo:], AF.Exp, scale=SCALE)
                    return pe

                pes = {}
                for pre in range(min(2, NB)):
                    pes[pre] = emit_sc_probs(pre)
                o_ps = l_ps = None
                for bi in range(NB):
                    qc, jj = blocks[bi]
                    if jj == 0:
                        o_ps = [
                            psp.tile([128, DV], F32, name=f"o_ps{i}", tag=f"o{i}")
                            for i in range(NDV)
                        ]
                        l_ps = psp.tile([128, 1], F32, name="l_ps", tag="l0")
                    if bi + 2 < NB:
                        pes[bi + 2] = emit_sc_probs(bi + 2)
                    pe = pes.pop(bi)
                    last = jj == C[qc] - 1
                    nc.tensor.matmul(
                        l_ps[:],
                        lhsT=pe[:],
                        rhs=ones[:],
                        start=(jj == 0),
                        stop=last,
                    )
                    for dvc in range(NDV):
                        nc.tensor.matmul(
                            o_ps[dvc][:],
                            lhsT=pe[:],
                            rhs=v_sb[:, jj * D + dvc * DV: jj * D + dvc * DV + DV],
                            start=(jj == 0),
                            stop=last,
                        )
                    if last:
                        # Clamp l away from 0 so fully-masked rows yield
                        # 0 (matching the reference), not 0 * inf = NaN.
                        rc = rcp.tile([128, 1], F32, name="rc", tag="rc")
                        lc = rcp.tile([128, 1], F32, name="lc", tag="lc")
                        nc.vector.tensor_scalar_max(lc[:], l_ps[:], 1e-30)
                        nc.vector.reciprocal(rc[:], lc[:])
                        if bi == NB - 1:
                            # Final drain is pure latency: split so the
                            # first half's DMA issues while the second
                            # half normalizes.
                            for dvc in range(NDV):
                                oth = otp.tile([128, DV], BF16, name="oth", tag="oth")
                                if dvc == 0:
                                    nc.vector.tensor_scalar_mul(
                                        oth[:], o_ps[0][:], rc[:]
                                    )
                                else:
                                    nc.scalar.activation(
                                        oth[:], o_ps[1][:], AF.Copy, scale=rc[:],
                                    )
                                nc.sync.dma_start(
                                    out[qc * QW: qc * QW + QW, dvc * DV: dvc * DV + DV],
                                    oth[:],
                                )
                        else:
                            ot = otp.tile([128, D], BF16, name="ot", tag="ot")
                            nc.vector.tensor_scalar_mul(ot[:, :DV], o_ps[0][:], rc[:])
                            nc.scalar.activation(
                                ot[:, DV:], o_ps[1][:], AF.Copy, scale=rc[:],
                            )
                            nc.sync.dma_start(
                                out[qc * QW: qc * QW + QW, :], ot[:]
                            )
    _elide_transitive_waits(nc)
    return nc


def _elide_transitive_waits(nc):
    """Drop semaphore waits already implied transitively.

    Hardware matmul (fused LDWEIGHTS) and DMA instruction encodings accept
    only ONE sync wait.  Tile's wait assignment is per-proc minimal but NOT
    transitive, so phase boundaries emit multi-wait matmuls/DMAs.  This pass
    walks the scheduled program (list order is a valid linearization),
    maintains a transitive vector clock per proc (engines and DMA queues are
    each FIFO), and removes waits that are (a) on the instruction's own proc
    (FIFO completion order), or (b) already implied by an earlier retained
    wait's transitive closure.
    """
    import re
    _proc_re = re.compile(r"^(PE|DVE|ACT|Act|Activation|SP|Pool|POOL|DMAHW\d+|DMASW\d+)_")

    def _is_proc_sem(name):
        return bool(_proc_re.match(name or ""))

    hist = {}      # sem id -> list of (tick, snapshot dict)
    state = {}     # proc key -> dict(sem id -> observed tick)
    tickc = {}     # sem id -> cumulative tick

    def snap_at(sem, t):
        h = hist.get(sem)
        if not h:
            return None
        lo, hi, best = 0, len(h) - 1, None
        while lo <= hi:
            mid = (lo + hi) // 2
            if h[mid][0] <= t:
                best = h[mid][1]
                lo = mid + 1
            else:
                hi = mid - 1
        return best

    splits = []
    for blk in nc.m.functions[0].blocks:
        for idx, i in enumerate(blk.instructions):
            si = i.sync_info
            if si is None:
                continue
            ups = [u for u in si.on_update if _is_proc_sem(u.ant_name)]
            own = ups[0].id if ups else ("eng", str(i.engine))
            v = state.setdefault(own, {})
            keep = []
            for w in list(si.on_wait):
                if (
                    w.wait_mode != "sem-ge-imm"
                    or w.wait_reg is not None
                    or not _is_proc_sem(w.ant_name)
                ):
                    keep.append(w)
                    continue
                # Same-proc elision is ONLY safe for PE matmuls: the PE
                # completes matmuls strictly in order (pc-monotone ends), so
                # a PE-self completion wait is redundant.  Other engines have
                # deep pipelines where same-engine WAR/WAW needs the wait.
                pe_self = (
                    w.id == own
                    and type(i).__name__ in ("InstMatmult", "InstLdweights")
                    and w.ant_name.startswith("PE")
                )
                if pe_self or v.get(w.id, 0) >= w.wait_value:
                    continue  # implied: PE FIFO or transitive closure
                keep.append(w)
                v[w.id] = max(v.get(w.id, 0), w.wait_value)
                s = snap_at(w.id, w.wait_value)
                if s:
                    for k2, t2 in s.items():
                        if v.get(k2, 0) < t2:
                            v[k2] = t2
            if len(keep) > 1 and all(_is_proc_sem(w.ant_name) for w in keep):
                # Hardware instruction encodings here accept at most one
                # sync wait: hoist all waits onto standalone sequencer
                # event-semaphore wait ops inserted just before.
                for k, w in enumerate(keep):
                    splits.append(
                        (blk, idx, mybir.InstEventSemaphore(
                            name=f"{i.name}-w{k}",
                            engine=i.engine,
                            sync_info=mybir.SyncInfo(on_wait=[w], on_update=[]),
                        ))
                    )
                keep = []
            if len(keep) != len(si.on_wait):
                si.on_wait = keep
                i.sync_info = si
            for u in ups:
                inc = u.update_value if u.update_mode in ("sem-inc", "sem-add-imm") else 0
                t = tickc.get(u.id, 0) + (inc or 0)
                tickc[u.id] = t
                snapshot = dict(v)
                snapshot[u.id] = t
                hist.setdefault(u.id, []).append((t, snapshot))
            nm = type(i).__name__
            if nm in ("InstMatmult", "InstDMACopy", "InstTensorCopy",
                      "InstTensorTensor", "InstActivation", "InstMemset",
                      "InstTensorScalarPtr", "InstReciprocal", "InstLdweights"):
                assert len(i.sync_info.on_wait) <= 1, (
                    i.name, nm,
                    [(w.ant_name, w.wait_value) for w in i.sync_info.on_wait],
                )
    by_blk = {}
    for blk, idx, inst in splits:
        by_blk.setdefault(id(blk), (blk, []))[1].append((idx, inst))
    for blk, items in by_blk.values():
        for idx, inst in sorted(items, key=lambda t: -t[0]):
            nc.register_instruction(inst)
            blk.instructions.insert(idx, inst)


_CACHE = {}


def _get_nc(structure=None):
    if structure is None:
        structure = _CACHE.get("struct", _CAUSAL)
    key = ("nc", structure)
    if key not in _CACHE:
        _CACHE[key] = _build_nc(structure)
    return _CACHE[key]


def make_in_maps(x, mask, Wq, Wk, Wv):
    x = np.asarray(x, dtype=np.float32)
    mask = np.asarray(mask)
    structure = _mask_structure(mask)
    _CACHE["struct"] = structure
    mix = _mix_order(structure)
    Wq = np.asarray(Wq, np.float32)
    Wk = np.asarray(Wk, np.float32)
    Wv = np.asarray(Wv, np.float32)
    # Weight layouts (shared by all cores), bf16:
    #   wq3/wk3[dc, p, mc*128+c] = W[dc*128+c, mc*128+p]
    #   wv3[dvc, p, mc*DV+c]     = Wv[dvc*DV+c, mc*128+p]
    wq3 = np.ascontiguousarray(
        Wq.reshape(NDC, 128, NMC, 128).transpose(0, 3, 2, 1).reshape(NDC, 128, NMC * 128)
    ).astype(BF)
    wk3 = np.ascontiguousarray(
        Wk.reshape(NDC, 128, NMC, 128).transpose(0, 3, 2, 1).reshape(NDC, 128, NMC * 128)
    ).astype(BF)
    wv3 = np.ascontiguousarray(
        Wv.reshape(NDV, DV, NMC, 128).transpose(0, 3, 2, 1).reshape(NDV, 128, NMC * DV)
    ).astype(BF)
    # xkv3[kc, p, mc*256+c] = x[b, kc*256+c, mc*128+p]  (per batch)
    xkv_b = {}
    for b in range(B):
        xkv_b[b] = np.ascontiguousarray(
            x[b].reshape(SKC, 256, NMC, 128).transpose(0, 3, 2, 1).reshape(SKC, 128, NMC * 256)
        ).astype(BF)
    in_maps = []
    for c in range(8):
        b, h = divmod(c, 2)
        rows = _rows(h)
        # xq3[mc, p, q] = x[b, rows[q], mc*128+p]
        xq3 = np.ascontiguousarray(
            x[b][rows].T.reshape(NMC, 128, NQ)
        ).astype(BF)
        mb = mask[b][rows]  # [1024 q, 2048 k]
        if mix:
            mmh = np.concatenate(
                [
                    mb[qc * QW:(qc + 1) * QW, j * 128:(j + 1) * 128].T
                    for (qc, j) in mix
                ],
                axis=1,
            ).astype(BF)
        else:
            mmh = np.zeros((128, QW), BF)
        in_maps.append(
            dict(
                xq3=xq3,
                xkv3=xkv_b[b],
                wq3=wq3,
                wk3=wk3,
                wv3=wv3,
                mmh=np.ascontiguousarray(mmh),
            )
        )
    return in_maps


def assemble(results):
    out = np.empty((B, S, D), np.float32)
    for c in range(8):
        b, h = divmod(c, 2)
        out[b, _rows(h)] = results[c]["out"]
    return out


def expected_core_out(expected, core):
    b, h = divmod(core, 2)
    return np.asarray(expected)[b][_rows(h)]


def kernel(x, mask, Wq, Wk, Wv):
    in_maps = make_in_maps(x, mask, Wq, Wk, Wv)
    nc = _get_nc(_CACHE["struct"])
    res = run_bass_kernel_spmd(nc, in_maps, list(range(8)))
    return assemble(res.results)
